# revision 43
# baseline (speedup 1.0000x reference)
"""Trainium2 Bass kernel for GrapherModule:
fc1+BN -> KNN(k=9) -> MaxRelative conv+BN+GELU -> fc2+BN -> +residual.

Sharding: 8 cores; core d handles batch b=d//4, query slice qoff=(d%4)*2048.
x is np.roll'ed by -qoff per core so queries are local nodes 0..2047 and the
SPMD program is identical on every core.

Single fused kernel per core:
  A) fc1 over own batch; other batch's BN1 stats via Gram matrix (no
     collective); h16 = fp16 normalized h.
  B) negx2 hi/lo fp16 rows, token-major h to HBM (PE fp16 transposes).
  C) per 128-query tile: scores = h16q^T @ h16 + negx2 (PSUM-seeded K=2
     matmul), Act copies PSUM->fp16 SBUF, diagonal self-mask, max8/max_index
     full-row top-8, on-device dma_gather of 9 neighbor feature rows
     (feature-major), gpsimd max-tree -> maxn, fused MaxRel conv matmuls.
  D) BNc AllReduce -> GELU -> fc2 -> BN2 AllReduce -> +residual -> y.
"""
import sys, os
sys.path.insert(0, '/opt/trn_rl_repo')
os.environ.setdefault('JAX_PLATFORMS', 'cpu')

import numpy as np

B, N, C = 2, 8192, 128
K = 9
NQ = 2048          # queries per core
NT = NQ // 128     # 16 query tiles per core
EPS = 1e-5
KDBG = int(os.environ.get('KDBG', '0'))

_CACHE = {}


def _build():
    import concourse.bass as bass
    import concourse.mybir as mybir
    import concourse.tile as tile
    from concourse import bacc
    from concourse.masks import make_identity

    dt = mybir.dt
    AF = mybir.ActivationFunctionType
    ALU = mybir.AluOpType
    AX = mybir.AxisListType
    f32r = dt.float32r

    nc = bacc.Bacc("TRN2", target_bir_lowering=False, debug=False,
                   enable_asserts=False, num_devices=8)

    # ---- I/O ----
    x_ownT = nc.dram_tensor("x_ownT", [C, N], dt.float32, kind="ExternalInput")
    x_res = nc.dram_tensor("x_res", [NQ, C], dt.float32, kind="ExternalInput")
    x_oth = nc.dram_tensor("x_oth", [N, C], dt.float32, kind="ExternalInput")
    fc1wT = nc.dram_tensor("fc1wT", [C, C], dt.float32, kind="ExternalInput")
    fc1wN = nc.dram_tensor("fc1wN", [C, C], dt.float32, kind="ExternalInput")
    fc1_b = nc.dram_tensor("fc1_b", [C], dt.float32, kind="ExternalInput")
    bn1_g = nc.dram_tensor("bn1_g", [C], dt.float32, kind="ExternalInput")
    bn1_b = nc.dram_tensor("bn1_b", [C], dt.float32, kind="ExternalInput")
    cwAT16 = nc.dram_tensor("cwAT16", [C, C], dt.float16, kind="ExternalInput")
    cw2T16 = nc.dram_tensor("cw2T16", [C, C], dt.float16, kind="ExternalInput")
    conv_b = nc.dram_tensor("conv_b", [C], dt.float32, kind="ExternalInput")
    bnc_g = nc.dram_tensor("bnc_g", [C], dt.float32, kind="ExternalInput")
    bnc_b = nc.dram_tensor("bnc_b", [C], dt.float32, kind="ExternalInput")
    fc2wT16 = nc.dram_tensor("fc2wT16", [C, C], dt.float16, kind="ExternalInput")
    fc2_b = nc.dram_tensor("fc2_b", [C], dt.float32, kind="ExternalInput")
    bn2_g = nc.dram_tensor("bn2_g", [C], dt.float32, kind="ExternalInput")
    bn2_b = nc.dram_tensor("bn2_b", [C], dt.float32, kind="ExternalInput")
    y = nc.dram_tensor("y", [NQ, C], dt.float32, kind="ExternalOutput")
    idx_d = nc.dram_tensor("idx_d", [NT * 128, K], dt.uint16, kind="Internal")
    repP = nc.dram_tensor("repP", [16, C], dt.float32, kind="ExternalInput")

    def col(t):  # [C] dram -> [C,1] view
        return t[:].rearrange("(c one) -> c one", one=1)

    with tile.TileContext(nc) as tc:
        wpool = tc.alloc_tile_pool(name="w", bufs=1)
        pers = tc.alloc_tile_pool(name="pers", bufs=1)
        dram = tc.alloc_tile_pool(name="dram", bufs=2, space="DRAM")

        identF = wpool.tile([128, 128], dt.float32)
        make_identity(nc, identF[:])
        identH = wpool.tile([128, 128], dt.float16)
        make_identity(nc, identH[:])
        ones16two = wpool.tile([2, 128], dt.float16)   # seed lhsT (K=2: hi+lo)
        nc.vector.memset(ones16two[:], 1.0)
        ones16col = wpool.tile([128, 1], dt.float16)   # negx2 lhsT (M=1)
        nc.vector.memset(ones16col[:], 1.0)

        ones32col = wpool.tile([128, 1], dt.float32)
        nc.vector.memset(ones32col[:], 1.0)
        repPs = wpool.tile([16, C], dt.float32)
        nc.gpsimd.dma_start(repPs[:], repP[:])
        fc1w = wpool.tile([C, C], dt.float32)
        nc.gpsimd.dma_start(fc1w[:], fc1wT[:])
        fc1wn = wpool.tile([C, C], dt.float32)
        nc.gpsimd.dma_start(fc1wn[:], fc1wN[:])
        cwA = wpool.tile([C, C], dt.float16)
        nc.gpsimd.dma_start(cwA[:], cwAT16[:])
        cw2 = wpool.tile([C, C], dt.float16)
        nc.gpsimd.dma_start(cw2[:], cw2T16[:])
        fc2w = wpool.tile([C, C], dt.float16)
        nc.gpsimd.dma_start(fc2w[:], fc2wT16[:])
        fc1b = wpool.tile([C, 1], dt.float32); nc.gpsimd.dma_start(fc1b[:], col(fc1_b))
        bn1g = wpool.tile([C, 1], dt.float32); nc.gpsimd.dma_start(bn1g[:], col(bn1_g))
        bn1bb = wpool.tile([C, 1], dt.float32); nc.gpsimd.dma_start(bn1bb[:], col(bn1_b))
        convb = wpool.tile([C, 1], dt.float32); nc.gpsimd.dma_start(convb[:], col(conv_b))
        bncg = wpool.tile([C, 1], dt.float32); nc.gpsimd.dma_start(bncg[:], col(bnc_g))
        bncb = wpool.tile([C, 1], dt.float32); nc.gpsimd.dma_start(bncb[:], col(bnc_b))
        fc2b = wpool.tile([C, 1], dt.float32); nc.gpsimd.dma_start(fc2b[:], col(fc2_b))
        bn2g = wpool.tile([C, 1], dt.float32); nc.gpsimd.dma_start(bn2g[:], col(bn2_g))
        bn2bb = wpool.tile([C, 1], dt.float32); nc.gpsimd.dma_start(bn2bb[:], col(bn2_b))

        # ---------- collective helpers ----------
        def allreduce2(sump, ssqp):
            loc = pers.tile([128, 2], dt.float32)
            nc.vector.reduce_sum(loc[:, 0:1], sump[:], axis=AX.X)
            nc.vector.reduce_sum(loc[:, 1:2], ssqp[:], axis=AX.X)
            bin_ = dram.tile([128, 2], dt.float32)
            bout = dram.tile([128, 2], dt.float32)
            nc.gpsimd.dma_start(bin_[:], loc[:])
            nc.gpsimd.collective_compute(
                "AllReduce", ALU.add, replica_groups=[list(range(8))],
                ins=[bin_.opt()], outs=[bout.opt()])
            tot = pers.tile([128, 2], dt.float32)
            nc.gpsimd.dma_start(tot[:], bout[:])
            return tot

        def bnparams(sumc, ssqc, gam, bet, inv_n):
            st = pers.tile([128, 8], dt.float32)
            mm, e2, vv, rr, sc, bi = (st[:, i:i + 1] for i in range(6))
            nc.vector.tensor_scalar_mul(mm, sumc, inv_n)
            nc.vector.tensor_scalar_mul(e2, ssqc, inv_n)
            nc.vector.tensor_tensor(vv, mm, mm, op=ALU.mult)
            nc.vector.tensor_sub(vv, e2, vv)
            nc.vector.tensor_scalar(vv, vv, EPS, None, op0=ALU.add)
            nc.vector.reciprocal(rr, vv)
            nc.scalar.activation(rr, rr, AF.Sqrt)
            nc.vector.tensor_tensor(sc, rr, gam, op=ALU.mult)
            nc.vector.tensor_tensor(bi, mm, sc, op=ALU.mult)
            nc.vector.tensor_sub(bi, bet, bi)
            return sc, bi

        # ---------- Phase A: fc1 over own batch, fp16 h + BN1 stats ----------
        hpre16 = pers.tile([128, N], dt.float16)
        s1 = pers.tile([128, 16], dt.float32)
        s2 = pers.tile([128, 16], dt.float32)

        # BN1 stats of the OTHER batch without computing its fc1:
        #   sum_oth = W @ (sum_n x_n) + N*b
        #   ssq_oth = diag(W G W^T) + 2 b * (W sum x) + N*b^2,  G = X^T X
        with tc.tile_pool(name="phA", bufs=3) as phA, \
             tc.tile_pool(name="go", bufs=3) as go, \
             tc.tile_pool(name="psM", bufs=2, space="PSUM") as psM, \
             tc.tile_pool(name="psG", bufs=1, space="PSUM") as psG, \
             tc.tile_pool(name="psg2", bufs=1, space="PSUM") as psg2:
            pg = psG.tile([128, 128], dt.float32)
            pxs = psg2.tile([128, 1], dt.float32)
            for t in range(16):
                r0 = t * 512
                xT = phA.tile([128, 512], dt.float32, tag="xT")
                nc.sync.dma_start(xT[:], x_ownT[:, r0:r0 + 512])
                pm = psM.tile([128, 512], dt.float32, tag="pm")
                nc.tensor.matmul(pm[:], fc1w[:], xT[:], start=True, stop=True)
                nc.scalar.activation(hpre16[:, r0:r0 + 512], pm[:], AF.Identity,
                                     bias=fc1b[:], accum_out=s1[:, t:t + 1])
                jk = phA.tile([128, 512], dt.float16, tag="jk")
                nc.scalar.activation(jk[:], hpre16[:, r0:r0 + 512], AF.Square,
                                     accum_out=s2[:, t:t + 1])
                xo = go.tile([128, 512], dt.float32, tag="xo")
                nc.sync.dma_start(
                    xo[:].rearrange("p (j c) -> p j c", j=4),
                    x_oth[r0:r0 + 512, :].rearrange("(j p) c -> p j c", p=128))
                for j in range(4):
                    xsl = xo[:, j * 128:(j + 1) * 128]
                    first = (t == 0 and j == 0)
                    last = (t == 15 and j == 3)
                    nc.tensor.matmul(pg[:], xsl, xsl, start=first, stop=last)
                    nc.tensor.matmul(pxs[:], xsl, ones32col[:],
                                     start=first, stop=last)
            gs = pers.tile([128, 128], dt.float32)
            nc.scalar.activation(gs[:], pg[:], AF.Copy)
            sxs = pers.tile([128, 1], dt.float32)
            nc.vector.tensor_copy(sxs[:], pxs[:])
            # t1 = W @ G ; sv = W @ sum_x
            pt1 = psG.tile([128, 128], dt.float32)
            nc.tensor.matmul(pt1[:], fc1w[:], gs[:], start=True, stop=True)
            t1s = pers.tile([128, 128], dt.float32)
            nc.scalar.activation(t1s[:], pt1[:], AF.Copy)
            psv = psg2.tile([128, 1], dt.float32)
            nc.tensor.matmul(psv[:], fc1w[:], sxs[:], start=True, stop=True)
            svs = pers.tile([128, 1], dt.float32)
            nc.vector.tensor_copy(svs[:], psv[:])
            # d = rowsum(t1 * W)
            tw = pers.tile([128, 128], dt.float32)
            nc.vector.tensor_tensor(tw[:], t1s[:], fc1wn[:], op=ALU.mult)
            stat_o = pers.tile([128, 6], dt.float32)
            doth = stat_o[:, 0:1]
            nc.vector.reduce_sum(doth, tw[:], axis=AX.X)
            sum_o, ssq_o, tmp_a, tmp_b = (stat_o[:, i:i + 1] for i in range(1, 5))
            nc.vector.scalar_tensor_tensor(sum_o, fc1b[:], float(N), svs[:],
                                           op0=ALU.mult, op1=ALU.add)
            nc.vector.tensor_tensor(tmp_a, fc1b[:], svs[:], op=ALU.mult)
            nc.vector.scalar_tensor_tensor(ssq_o, tmp_a, 2.0, doth,
                                           op0=ALU.mult, op1=ALU.add)
            nc.vector.tensor_tensor(tmp_b, fc1b[:], fc1b[:], op=ALU.mult)
            nc.vector.scalar_tensor_tensor(ssq_o, tmp_b, float(N), ssq_o,
                                           op0=ALU.mult, op1=ALU.add)
            # combine with own-batch accumulators
            tot1 = pers.tile([128, 2], dt.float32)
            nc.vector.reduce_sum(tot1[:, 0:1], s1[:], axis=AX.X)
            nc.vector.reduce_sum(tot1[:, 1:2], s2[:], axis=AX.X)
            nc.vector.tensor_add(tot1[:, 0:1], tot1[:, 0:1], sum_o)
            nc.vector.tensor_add(tot1[:, 1:2], tot1[:, 1:2], ssq_o)

        sc1, bi1 = bnparams(tot1[:, 0:1], tot1[:, 1:2], bn1g[:], bn1bb[:],
                            1.0 / (B * N))
        h32 = pers.tile([128, N], dt.float32)
        h16 = pers.tile([128, N], dt.float16)
        for cch in range(4):
            sl = slice(cch * 2048, (cch + 1) * 2048)
            nc.scalar.activation(h32[:, sl], hpre16[:, sl], AF.Identity,
                                 bias=bi1, scale=sc1)
            nc.vector.tensor_copy(h16[:, sl], h32[:, sl])

        # ---------- negx2 row (f32) + token-major h to HBM ----------
        negx2 = pers.tile([2, N], dt.float16)   # hi + lo split of -0.5*|h|^2
        nxhi = pers.tile([1, N], dt.float16)
        nxlo = pers.tile([1, N], dt.float16)
        with tc.tile_pool(name="nx", bufs=3) as nx, \
             tc.tile_pool(name="psN", bufs=2, space="PSUM") as psN:
            for cch in range(16):
                sl = slice(cch * 512, (cch + 1) * 512)
                hh = nx.tile([128, 512], dt.float16, tag="hh")
                nc.vector.tensor_tensor(hh[:], h16[:, sl], h16[:, sl], op=ALU.mult)
                pn = psN.tile([128, 512], dt.float32, tag="pn")
                nc.tensor.matmul(pn[0:1, :], ones16col[:], hh[:],
                                 start=True, stop=True)
                nc.scalar.activation(nxhi[0:1, sl], pn[0:1, :], AF.Copy,
                                     scale=-0.5)
                nc.vector.scalar_tensor_tensor(nxlo[0:1, sl], pn[0:1, :], -0.5,
                                               nxhi[0:1, sl],
                                               op0=ALU.mult, op1=ALU.subtract)
            for cch in range(4):
                sl = slice(cch * 2048, (cch + 1) * 2048)
                nc.sync.dma_start(negx2[0:1, sl], nxhi[0:1, sl])
                nc.sync.dma_start(negx2[1:2, sl], nxlo[0:1, sl])

        # ---------- Phase B/C: scores, top-8, gather, MaxRel conv ----------
        convpre = pers.tile([128, NQ], dt.float32)
        cs1 = pers.tile([128, NT], dt.float32)
        cs2 = pers.tile([128, NT], dt.float32)

        iota16 = pers.tile([128, NT], dt.uint16)   # col t = t*128 + partition
        nc.gpsimd.iota(iota16[:], pattern=[[128, NT]], base=0,
                       channel_multiplier=1)

        with tc.tile_pool(name="stp", bufs=2) as stp, \
             tc.tile_pool(name="sm", bufs=4) as sm, \
             tc.tile_pool(name="gth", bufs=2) as gth, \
             tc.tile_pool(name="mxp", bufs=2) as mxp, \
             tc.tile_pool(name="psS", bufs=3, space="PSUM") as psS, \
             tc.tile_pool(name="psC", bufs=1, space="PSUM") as psC, \
             tc.tile_pool(name="psR", bufs=1, space="PSUM") as psR:
            for i in range(NT):
                q0 = i * 128
                st = stp.tile([128, N], dt.float16, tag="st")
                for c8 in range(8):
                    sl = slice(c8 * 1024, (c8 + 1) * 1024)
                    ps = psS.tile([128, 1024], dt.float32, tag="ps")
                    for hb_ in range(2):
                        psl = slice(hb_ * 512, (hb_ + 1) * 512)
                        nsl = slice(c8 * 1024 + hb_ * 512, c8 * 1024 + (hb_ + 1) * 512)
                        nc.tensor.matmul(ps[:, psl], ones16two[:],
                                         negx2[0:2, nsl],
                                         start=True, stop=False)
                        nc.tensor.matmul(ps[:, psl], h16[:, q0:q0 + 128],
                                         h16[:, nsl], start=False, stop=True)
                    nc.scalar.activation(st[:, sl], ps[:], AF.Identity)
                nc.gpsimd.affine_select(
                    st[:, q0:q0 + 128], st[:, q0:q0 + 128],
                    pattern=[[1, 128]], compare_op=ALU.not_equal,
                    fill=-60000.0, base=0, channel_multiplier=-1)
                top8 = sm.tile([128, 8], dt.float16, tag="t8")
                nc.vector.max(top8[:], st[:])
                idx9 = sm.tile([128, K], dt.uint16, tag="i9")
                nc.vector.tensor_copy(idx9[:, 0:1], iota16[:, i:i + 1])
                nc.vector.max_index(idx9[:, 1:K], top8[:], st[:])
                # dram bounce -> wrapped 16-partition idx layout
                nc.sync.dma_start(idx_d[q0:q0 + 128, :], idx9[:])
                ta = mxp.tile([128, 128], dt.float16, tag="ta")
                if KDBG == 1:
                    nc.vector.tensor_copy(ta[:], h16[:, q0:q0 + 128])
                elif KDBG == 2:
                    idxw = sm.tile([128, 72], dt.uint16, tag="iw")
                    nc.gpsimd.memset(idxw[:], 0)
                    nc.sync.dma_start(
                        idxw[0:16, :].rearrange("p (j g) -> p j g", j=K),
                        idx_d[q0:q0 + 128, :].rearrange("(g p) j -> p j g", p=16))
                    jw = sm.tile([128, 72], dt.uint16, tag="jw")
                    nc.vector.tensor_copy(jw[:], idxw[:])  # consume idxw
                    nc.vector.tensor_copy(ta[:], h16[:, q0:q0 + 128])
                else:
                    idxw16 = sm.tile([16, 72], dt.uint16, tag="iw16")
                    nc.sync.dma_start(
                        idxw16[:].rearrange("p (j g) -> p j g", j=K),
                        idx_d[q0:q0 + 128, :].rearrange("(g p) j -> p j g", p=16))
                    idxwf = sm.tile([16, 72], dt.float32, tag="iwf")
                    nc.scalar.activation(idxwf[:], idxw16[:], AF.Copy)
                    pr = psR.tile([128, 72], dt.float32, tag="pr")
                    nc.tensor.matmul(pr[:], repPs[:], idxwf[:],
                                     start=True, stop=True)
                    idxw = sm.tile([128, 72], dt.int16, tag="iw")
                    nc.scalar.activation(idxw[:], pr[:], AF.Copy)
                    gat = gth.tile([128, K * 128], dt.float32, tag="gat")
                    nc.gpsimd.ap_gather(
                        gat[:].rearrange("c (n d) -> c n d", d=1),
                        h32[:].rearrange("c (n d) -> c n d", d=1),
                        idxw[:], channels=128, num_elems=N, d=1,
                        num_idxs=K * 128)
                    # maxn over the 9 gathered slices (wide max tree, f32)
                    fa = mxp.tile([128, 512], dt.float32, tag="fa")
                    nc.vector.tensor_tensor(fa[:], gat[:, 0:512],
                                            gat[:, 512:1024], op=ALU.max)
                    fb = mxp.tile([128, 256], dt.float32, tag="fb")
                    nc.vector.tensor_tensor(fb[:], fa[:, 0:256], fa[:, 256:512],
                                            op=ALU.max)
                    fc = mxp.tile([128, 128], dt.float32, tag="fc")
                    nc.vector.tensor_tensor(fc[:], fb[:, 0:128], fb[:, 128:256],
                                            op=ALU.max)
                    nc.vector.tensor_tensor(ta[:], fc[:], gat[:, 1024:1152],
                                            op=ALU.max)
                # MaxRel conv: convpre = A @ h_q + W2 @ maxn  (A = W1 - W2)
                pc = psC.tile([128, 128], dt.float32, tag="pc")
                nc.tensor.matmul(pc[:], cwA[:], h16[:, q0:q0 + 128],
                                 start=True, stop=False)
                nc.tensor.matmul(pc[:], cw2[:], ta[:], start=False, stop=True)
                nc.scalar.activation(convpre[:, q0:q0 + 128], pc[:], AF.Identity,
                                     bias=convb[:], accum_out=cs1[:, i:i + 1])
                jc = sm.tile([128, 128], dt.float16, tag="jc")
                nc.scalar.activation(jc[:], convpre[:, q0:q0 + 128], AF.Square,
                                     accum_out=cs2[:, i:i + 1])

        # ---------- Phase D: BNc -> GELU -> fc2 -> BN2 -> +residual ----------
        totc = allreduce2(cs1, cs2)
        scc, bic = bnparams(totc[:, 0:1], totc[:, 1:2], bncg[:], bncb[:],
                            1.0 / (B * N))
        g16 = pers.tile([128, NQ], dt.float16)
        for cch in range(4):
            sl = slice(cch * 512, (cch + 1) * 512)
            nc.scalar.activation(g16[:, sl], convpre[:, sl], AF.Gelu,
                                 bias=bic, scale=scc)

        f2pre = pers.tile([128, NQ], dt.float32)
        fs1 = pers.tile([128, 4], dt.float32)
        fs2 = pers.tile([128, 4], dt.float32)
        with tc.tile_pool(name="fj", bufs=2) as fj, \
             tc.tile_pool(name="psF", bufs=2, space="PSUM") as psF:
            for cch in range(4):
                sl = slice(cch * 512, (cch + 1) * 512)
                pf = psF.tile([128, 512], dt.float32, tag="pf")
                nc.tensor.matmul(pf[:], fc2w[:], g16[:, sl], start=True, stop=True)
                nc.scalar.activation(f2pre[:, sl], pf[:], AF.Identity,
                                     bias=fc2b[:], accum_out=fs1[:, cch:cch + 1])
                jf = fj.tile([128, 512], dt.float16, tag="jf")
                nc.vector.scalar_tensor_tensor(jf[:], f2pre[:, sl], 1.0,
                                               f2pre[:, sl], op0=ALU.mult,
                                               op1=ALU.mult,
                                               accum_out=fs2[:, cch:cch + 1])

        totf = allreduce2(fs1, fs2)
        scf, bif = bnparams(totf[:, 0:1], totf[:, 1:2], bn2g[:], bn2bb[:],
                            1.0 / (B * N))
        outfm = pers.tile([128, NQ], dt.float32)
        for cch in range(4):
            sl = slice(cch * 512, (cch + 1) * 512)
            nc.scalar.activation(outfm[:, sl], f2pre[:, sl], AF.Identity,
                                 bias=bif, scale=scf)

        with tc.tile_pool(name="op", bufs=3) as op, \
             tc.tile_pool(name="psO", bufs=2, space="PSUM") as psO:
            for t in range(4):
                r0 = t * 512
                xr = op.tile([128, 512], dt.float32, tag="xr")
                nc.sync.dma_start(
                    xr[:].rearrange("p (j c) -> p j c", j=4),
                    x_res[r0:r0 + 512, :].rearrange("(j p) c -> p j c", p=128))
                po = psO.tile([128, 512], dt.float32, tag="po")
                for j in range(4):
                    q0 = r0 + j * 128
                    nc.tensor.transpose(po[:, j * 128:(j + 1) * 128],
                                        outfm[:, q0:q0 + 128], identF[:])
                ot = op.tile([128, 512], dt.float32, tag="ot")
                nc.vector.tensor_add(ot[:], po[:], xr[:])
                nc.sync.dma_start(
                    y[r0:r0 + 512, :].rearrange("(j p) c -> p j c", p=128),
                    ot[:].rearrange("p (j c) -> p j c", j=4))

        for p in (dram, pers, wpool):
            p.release()

    nc.compile()
    return nc


def _prep_weights(inputs):
    f32 = lambda a: np.ascontiguousarray(np.asarray(a), dtype=np.float32)
    f16 = lambda a: np.ascontiguousarray(np.asarray(a), dtype=np.float16)
    conv_w = np.asarray(inputs['conv_w'], np.float32)
    w1 = conv_w[:, :C]
    w2 = conv_w[:, C:]
    return {
        'fc1wT': f32(np.asarray(inputs['fc1_w'], np.float32).T),
        'fc1wN': f32(inputs['fc1_w']),
        'fc1_b': f32(inputs['fc1_b']),
        'bn1_g': f32(inputs['bn1_g']),
        'bn1_b': f32(inputs['bn1_b']),
        'cwAT16': f16((w1 - w2).T),
        'cw2T16': f16(w2.T),
        'conv_b': f32(inputs['conv_b']),
        'bnc_g': f32(inputs['bnc_g']),
        'bnc_b': f32(inputs['bnc_b']),
        'fc2wT16': f16(np.asarray(inputs['fc2_w'], np.float32).T),
        'fc2_b': f32(inputs['fc2_b']),
        'bn2_g': f32(inputs['bn2_g']),
        'bn2_b': f32(inputs['bn2_b']),
        'repP': f32(np.equal(np.arange(C)[None, :] % 16,
                             np.arange(16)[:, None]).astype(np.float32)),
    }


def _in_maps(inputs):
    x = np.ascontiguousarray(np.asarray(inputs['x']), dtype=np.float32)
    w = _prep_weights(inputs)
    maps = []
    for d in range(8):
        b, qoff = d // 4, (d % 4) * NQ
        m = dict(w)
        xr = np.roll(x[b], -qoff, axis=0)
        m['x_ownT'] = np.ascontiguousarray(xr.T)
        m['x_res'] = np.ascontiguousarray(x[b, qoff:qoff + NQ])
        m['x_oth'] = np.ascontiguousarray(x[1 - b])
        maps.append(m)
    return maps


def kernel(**inputs):
    from concourse import bass_utils

    if 'nc1' not in _CACHE:
        _CACHE['nc1'] = _build()
    nc1 = _CACHE['nc1']

    in_maps = _in_maps(inputs)
    r1 = bass_utils.run_bass_kernel_spmd(nc1, in_maps, core_ids=list(range(8)))
    _CACHE['last_res'] = r1

    out = np.empty((B, N, C), np.float32)
    for d in range(8):
        b, qoff = d // 4, (d % 4) * NQ
        out[b, qoff:qoff + NQ] = r1.results[d]['y']
    return out


# revision 44
# speedup vs baseline: 1.0639x; 1.0639x over previous
"""Trainium2 Bass kernel for GrapherModule:
fc1+BN -> KNN(k=9) -> MaxRelative conv+BN+GELU -> fc2+BN -> +residual.

Sharding: 8 cores; core d handles batch b=d//4, query slice qoff=(d%4)*2048.
x is np.roll'ed by -qoff per core so queries are local nodes 0..2047 and the
SPMD program is identical on every core.

Single fused kernel per core:
  A) fc1 over own batch; other batch's BN1 stats via Gram matrix (no
     collective); h16 = fp16 normalized h.
  B) negx2 = -0.5*|h16|^2 stored as fp16 hi+lo rows (split precision).
  C) per 128-query tile: scores = h16q^T @ h16 + negx2 (PSUM pre-seeded by a
     K=2 hi+lo rank-2 matmul), Act copies PSUM->fp16 SBUF, diagonal
     self-mask, full-row max8/max_index top-8, indices rearranged to the
     wrapped 16-partition layout via a DRAM bounce + PE replication matmul,
     on-device gpsimd ap_gather of the 9 neighbor columns from f32 h
     (feature-major), DVE max-tree -> maxn, fused MaxRel conv matmuls.
  D) BNc AllReduce -> GELU -> fc2 -> BN2 AllReduce -> +residual -> y.
  (dma_gather is avoided: it faults under this runtime; ap_gather with
   group-replicated indices is the working on-device gather.)
"""
import sys, os
sys.path.insert(0, '/opt/trn_rl_repo')
os.environ.setdefault('JAX_PLATFORMS', 'cpu')

import numpy as np

B, N, C = 2, 8192, 128
K = 9
NQ = 2048          # queries per core
NT = NQ // 128     # 16 query tiles per core
EPS = 1e-5
KDBG = int(os.environ.get('KDBG', '0'))

_CACHE = {}


def _build():
    import concourse.bass as bass
    import concourse.mybir as mybir
    import concourse.tile as tile
    from concourse import bacc
    from concourse.masks import make_identity

    dt = mybir.dt
    AF = mybir.ActivationFunctionType
    ALU = mybir.AluOpType
    AX = mybir.AxisListType
    f32r = dt.float32r

    nc = bacc.Bacc("TRN2", target_bir_lowering=False, debug=False,
                   enable_asserts=False, num_devices=8)

    # ---- I/O ----
    x_ownT = nc.dram_tensor("x_ownT", [C, N], dt.float32, kind="ExternalInput")
    x_res = nc.dram_tensor("x_res", [NQ, C], dt.float32, kind="ExternalInput")
    x_oth = nc.dram_tensor("x_oth", [N, C], dt.float32, kind="ExternalInput")
    fc1wT = nc.dram_tensor("fc1wT", [C, C], dt.float32, kind="ExternalInput")
    fc1wN = nc.dram_tensor("fc1wN", [C, C], dt.float32, kind="ExternalInput")
    fc1_b = nc.dram_tensor("fc1_b", [C], dt.float32, kind="ExternalInput")
    bn1_g = nc.dram_tensor("bn1_g", [C], dt.float32, kind="ExternalInput")
    bn1_b = nc.dram_tensor("bn1_b", [C], dt.float32, kind="ExternalInput")
    cwAT16 = nc.dram_tensor("cwAT16", [C, C], dt.float16, kind="ExternalInput")
    cw2T16 = nc.dram_tensor("cw2T16", [C, C], dt.float16, kind="ExternalInput")
    conv_b = nc.dram_tensor("conv_b", [C], dt.float32, kind="ExternalInput")
    bnc_g = nc.dram_tensor("bnc_g", [C], dt.float32, kind="ExternalInput")
    bnc_b = nc.dram_tensor("bnc_b", [C], dt.float32, kind="ExternalInput")
    fc2wT16 = nc.dram_tensor("fc2wT16", [C, C], dt.float16, kind="ExternalInput")
    fc2_b = nc.dram_tensor("fc2_b", [C], dt.float32, kind="ExternalInput")
    bn2_g = nc.dram_tensor("bn2_g", [C], dt.float32, kind="ExternalInput")
    bn2_b = nc.dram_tensor("bn2_b", [C], dt.float32, kind="ExternalInput")
    y = nc.dram_tensor("y", [NQ, C], dt.float32, kind="ExternalOutput")
    idx_d = nc.dram_tensor("idx_d", [NT * 128, K], dt.uint16, kind="Internal")
    repP = nc.dram_tensor("repP", [16, C], dt.float32, kind="ExternalInput")

    def col(t):  # [C] dram -> [C,1] view
        return t[:].rearrange("(c one) -> c one", one=1)

    with tile.TileContext(nc) as tc:
        wpool = tc.alloc_tile_pool(name="w", bufs=1)
        pers = tc.alloc_tile_pool(name="pers", bufs=1)
        dram = tc.alloc_tile_pool(name="dram", bufs=2, space="DRAM")

        identF = wpool.tile([128, 128], dt.float32)
        make_identity(nc, identF[:])
        identH = wpool.tile([128, 128], dt.float16)
        make_identity(nc, identH[:])
        ones16two = wpool.tile([2, 128], dt.float16)   # seed lhsT (K=2: hi+lo)
        nc.vector.memset(ones16two[:], 1.0)
        ones16col = wpool.tile([128, 1], dt.float16)   # negx2 lhsT (M=1)
        nc.vector.memset(ones16col[:], 1.0)

        ones32col = wpool.tile([128, 1], dt.float32)
        nc.vector.memset(ones32col[:], 1.0)
        repPs = wpool.tile([16, C], dt.float32)
        nc.gpsimd.dma_start(repPs[:], repP[:])
        fc1w = wpool.tile([C, C], dt.float32)
        nc.gpsimd.dma_start(fc1w[:], fc1wT[:])
        fc1wn = wpool.tile([C, C], dt.float32)
        nc.gpsimd.dma_start(fc1wn[:], fc1wN[:])
        cwA = wpool.tile([C, C], dt.float16)
        nc.gpsimd.dma_start(cwA[:], cwAT16[:])
        cw2 = wpool.tile([C, C], dt.float16)
        nc.gpsimd.dma_start(cw2[:], cw2T16[:])
        fc2w = wpool.tile([C, C], dt.float16)
        nc.gpsimd.dma_start(fc2w[:], fc2wT16[:])
        fc1b = wpool.tile([C, 1], dt.float32); nc.gpsimd.dma_start(fc1b[:], col(fc1_b))
        bn1g = wpool.tile([C, 1], dt.float32); nc.gpsimd.dma_start(bn1g[:], col(bn1_g))
        bn1bb = wpool.tile([C, 1], dt.float32); nc.gpsimd.dma_start(bn1bb[:], col(bn1_b))
        convb = wpool.tile([C, 1], dt.float32); nc.gpsimd.dma_start(convb[:], col(conv_b))
        bncg = wpool.tile([C, 1], dt.float32); nc.gpsimd.dma_start(bncg[:], col(bnc_g))
        bncb = wpool.tile([C, 1], dt.float32); nc.gpsimd.dma_start(bncb[:], col(bnc_b))
        fc2b = wpool.tile([C, 1], dt.float32); nc.gpsimd.dma_start(fc2b[:], col(fc2_b))
        bn2g = wpool.tile([C, 1], dt.float32); nc.gpsimd.dma_start(bn2g[:], col(bn2_g))
        bn2bb = wpool.tile([C, 1], dt.float32); nc.gpsimd.dma_start(bn2bb[:], col(bn2_b))

        # ---------- collective helpers ----------
        def allreduce2(sump, ssqp):
            loc = pers.tile([128, 2], dt.float32)
            nc.vector.reduce_sum(loc[:, 0:1], sump[:], axis=AX.X)
            nc.vector.reduce_sum(loc[:, 1:2], ssqp[:], axis=AX.X)
            bin_ = dram.tile([128, 2], dt.float32)
            bout = dram.tile([128, 2], dt.float32)
            nc.gpsimd.dma_start(bin_[:], loc[:])
            nc.gpsimd.collective_compute(
                "AllReduce", ALU.add, replica_groups=[list(range(8))],
                ins=[bin_.opt()], outs=[bout.opt()])
            tot = pers.tile([128, 2], dt.float32)
            nc.gpsimd.dma_start(tot[:], bout[:])
            return tot

        def bnparams(sumc, ssqc, gam, bet, inv_n):
            st = pers.tile([128, 8], dt.float32)
            mm, e2, vv, rr, sc, bi = (st[:, i:i + 1] for i in range(6))
            nc.vector.tensor_scalar_mul(mm, sumc, inv_n)
            nc.vector.tensor_scalar_mul(e2, ssqc, inv_n)
            nc.vector.tensor_tensor(vv, mm, mm, op=ALU.mult)
            nc.vector.tensor_sub(vv, e2, vv)
            nc.vector.tensor_scalar(vv, vv, EPS, None, op0=ALU.add)
            nc.vector.reciprocal(rr, vv)
            nc.scalar.activation(rr, rr, AF.Sqrt)
            nc.vector.tensor_tensor(sc, rr, gam, op=ALU.mult)
            nc.vector.tensor_tensor(bi, mm, sc, op=ALU.mult)
            nc.vector.tensor_sub(bi, bet, bi)
            return sc, bi

        # ---------- Phase A: fc1 over own batch, fp16 h + BN1 stats ----------
        hpre16 = pers.tile([128, N], dt.float16)
        s1 = pers.tile([128, 16], dt.float32)
        s2 = pers.tile([128, 16], dt.float32)

        # BN1 stats of the OTHER batch without computing its fc1:
        #   sum_oth = W @ (sum_n x_n) + N*b
        #   ssq_oth = diag(W G W^T) + 2 b * (W sum x) + N*b^2,  G = X^T X
        with tc.tile_pool(name="phA", bufs=3) as phA, \
             tc.tile_pool(name="go", bufs=3) as go, \
             tc.tile_pool(name="psM", bufs=2, space="PSUM") as psM, \
             tc.tile_pool(name="psG", bufs=1, space="PSUM") as psG, \
             tc.tile_pool(name="psg2", bufs=1, space="PSUM") as psg2:
            pg = psG.tile([128, 128], dt.float32)
            pxs = psg2.tile([128, 1], dt.float32)
            for t in range(16):
                r0 = t * 512
                xT = phA.tile([128, 512], dt.float32, tag="xT")
                nc.sync.dma_start(xT[:], x_ownT[:, r0:r0 + 512])
                pm = psM.tile([128, 512], dt.float32, tag="pm")
                nc.tensor.matmul(pm[:], fc1w[:], xT[:], start=True, stop=True)
                nc.scalar.activation(hpre16[:, r0:r0 + 512], pm[:], AF.Identity,
                                     bias=fc1b[:], accum_out=s1[:, t:t + 1])
                jk = phA.tile([128, 512], dt.float16, tag="jk")
                nc.scalar.activation(jk[:], hpre16[:, r0:r0 + 512], AF.Square,
                                     accum_out=s2[:, t:t + 1])
                xo = go.tile([128, 512], dt.float32, tag="xo")
                nc.sync.dma_start(
                    xo[:].rearrange("p (j c) -> p j c", j=4),
                    x_oth[r0:r0 + 512, :].rearrange("(j p) c -> p j c", p=128))
                for j in range(4):
                    xsl = xo[:, j * 128:(j + 1) * 128]
                    first = (t == 0 and j == 0)
                    last = (t == 15 and j == 3)
                    nc.tensor.matmul(pg[:], xsl, xsl, start=first, stop=last)
                    nc.tensor.matmul(pxs[:], xsl, ones32col[:],
                                     start=first, stop=last)
            gs = pers.tile([128, 128], dt.float32)
            nc.scalar.activation(gs[:], pg[:], AF.Copy)
            sxs = pers.tile([128, 1], dt.float32)
            nc.vector.tensor_copy(sxs[:], pxs[:])
            # t1 = W @ G ; sv = W @ sum_x
            pt1 = psG.tile([128, 128], dt.float32)
            nc.tensor.matmul(pt1[:], fc1w[:], gs[:], start=True, stop=True)
            t1s = pers.tile([128, 128], dt.float32)
            nc.scalar.activation(t1s[:], pt1[:], AF.Copy)
            psv = psg2.tile([128, 1], dt.float32)
            nc.tensor.matmul(psv[:], fc1w[:], sxs[:], start=True, stop=True)
            svs = pers.tile([128, 1], dt.float32)
            nc.vector.tensor_copy(svs[:], psv[:])
            # d = rowsum(t1 * W)
            tw = pers.tile([128, 128], dt.float32)
            nc.vector.tensor_tensor(tw[:], t1s[:], fc1wn[:], op=ALU.mult)
            stat_o = pers.tile([128, 6], dt.float32)
            doth = stat_o[:, 0:1]
            nc.vector.reduce_sum(doth, tw[:], axis=AX.X)
            sum_o, ssq_o, tmp_a, tmp_b = (stat_o[:, i:i + 1] for i in range(1, 5))
            nc.vector.scalar_tensor_tensor(sum_o, fc1b[:], float(N), svs[:],
                                           op0=ALU.mult, op1=ALU.add)
            nc.vector.tensor_tensor(tmp_a, fc1b[:], svs[:], op=ALU.mult)
            nc.vector.scalar_tensor_tensor(ssq_o, tmp_a, 2.0, doth,
                                           op0=ALU.mult, op1=ALU.add)
            nc.vector.tensor_tensor(tmp_b, fc1b[:], fc1b[:], op=ALU.mult)
            nc.vector.scalar_tensor_tensor(ssq_o, tmp_b, float(N), ssq_o,
                                           op0=ALU.mult, op1=ALU.add)
            # combine with own-batch accumulators
            tot1 = pers.tile([128, 2], dt.float32)
            nc.vector.reduce_sum(tot1[:, 0:1], s1[:], axis=AX.X)
            nc.vector.reduce_sum(tot1[:, 1:2], s2[:], axis=AX.X)
            nc.vector.tensor_add(tot1[:, 0:1], tot1[:, 0:1], sum_o)
            nc.vector.tensor_add(tot1[:, 1:2], tot1[:, 1:2], ssq_o)

        sc1, bi1 = bnparams(tot1[:, 0:1], tot1[:, 1:2], bn1g[:], bn1bb[:],
                            1.0 / (B * N))
        h32 = pers.tile([128, N], dt.float32)
        h16 = pers.tile([128, N], dt.float16)
        for cch in range(4):
            sl = slice(cch * 2048, (cch + 1) * 2048)
            nc.scalar.activation(h32[:, sl], hpre16[:, sl], AF.Identity,
                                 bias=bi1, scale=sc1)
            nc.vector.tensor_copy(h16[:, sl], h32[:, sl])

        # ---------- negx2 row (f32) + token-major h to HBM ----------
        negx2 = pers.tile([2, N], dt.float16)   # hi + lo split of -0.5*|h|^2
        nxhi = pers.tile([1, N], dt.float16)
        nxlo = pers.tile([1, N], dt.float16)
        with tc.tile_pool(name="nx", bufs=3) as nx, \
             tc.tile_pool(name="psN", bufs=2, space="PSUM") as psN:
            for cch in range(16):
                sl = slice(cch * 512, (cch + 1) * 512)
                hh = nx.tile([128, 512], dt.float16, tag="hh")
                nc.vector.tensor_tensor(hh[:], h16[:, sl], h16[:, sl], op=ALU.mult)
                pn = psN.tile([128, 512], dt.float32, tag="pn")
                nc.tensor.matmul(pn[0:1, :], ones16col[:], hh[:],
                                 start=True, stop=True)
                nc.scalar.activation(nxhi[0:1, sl], pn[0:1, :], AF.Copy,
                                     scale=-0.5)
                nc.vector.scalar_tensor_tensor(nxlo[0:1, sl], pn[0:1, :], -0.5,
                                               nxhi[0:1, sl],
                                               op0=ALU.mult, op1=ALU.subtract)
            for cch in range(4):
                sl = slice(cch * 2048, (cch + 1) * 2048)
                nc.sync.dma_start(negx2[0:1, sl], nxhi[0:1, sl])
                nc.sync.dma_start(negx2[1:2, sl], nxlo[0:1, sl])

        # ---------- Phase B/C: scores, top-8, gather, MaxRel conv ----------
        convpre = pers.tile([128, NQ], dt.float32)
        cs1 = pers.tile([128, NT], dt.float32)
        cs2 = pers.tile([128, NT], dt.float32)

        iota16 = pers.tile([128, NT], dt.uint16)   # col t = t*128 + partition
        nc.gpsimd.iota(iota16[:], pattern=[[128, NT]], base=0,
                       channel_multiplier=1)

        with tc.tile_pool(name="stp", bufs=2) as stp, \
             tc.tile_pool(name="sm", bufs=4) as sm, \
             tc.tile_pool(name="gth", bufs=2) as gth, \
             tc.tile_pool(name="mxp", bufs=2) as mxp, \
             tc.tile_pool(name="psS", bufs=3, space="PSUM") as psS, \
             tc.tile_pool(name="psC", bufs=1, space="PSUM") as psC, \
             tc.tile_pool(name="psR", bufs=1, space="PSUM") as psR:
            for i in range(NT):
                q0 = i * 128
                st = stp.tile([128, N], dt.float16, tag="st")
                for c8 in range(8):
                    sl = slice(c8 * 1024, (c8 + 1) * 1024)
                    ps = psS.tile([128, 1024], dt.float32, tag="ps")
                    for hb_ in range(2):
                        psl = slice(hb_ * 512, (hb_ + 1) * 512)
                        nsl = slice(c8 * 1024 + hb_ * 512, c8 * 1024 + (hb_ + 1) * 512)
                        nc.tensor.matmul(ps[:, psl], ones16two[:],
                                         negx2[0:2, nsl],
                                         start=True, stop=False)
                        nc.tensor.matmul(ps[:, psl], h16[:, q0:q0 + 128],
                                         h16[:, nsl], start=False, stop=True)
                    nc.scalar.activation(st[:, sl], ps[:], AF.Identity)
                nc.gpsimd.affine_select(
                    st[:, q0:q0 + 128], st[:, q0:q0 + 128],
                    pattern=[[1, 128]], compare_op=ALU.not_equal,
                    fill=-60000.0, base=0, channel_multiplier=-1)
                top8 = sm.tile([128, 8], dt.float16, tag="t8")
                nc.vector.max(top8[:], st[:])
                idx9 = sm.tile([128, K], dt.uint16, tag="i9")
                nc.vector.tensor_copy(idx9[:, 0:1], iota16[:, i:i + 1])
                nc.vector.max_index(idx9[:, 1:K], top8[:], st[:])
                # dram bounce -> wrapped 16-partition idx layout
                nc.sync.dma_start(idx_d[q0:q0 + 128, :], idx9[:])
                ta = mxp.tile([128, 128], dt.float16, tag="ta")
                if KDBG == 1:
                    nc.vector.tensor_copy(ta[:], h16[:, q0:q0 + 128])
                elif KDBG == 2:
                    idxw = sm.tile([128, 72], dt.uint16, tag="iw")
                    nc.gpsimd.memset(idxw[:], 0)
                    nc.sync.dma_start(
                        idxw[0:16, :].rearrange("p (j g) -> p j g", j=K),
                        idx_d[q0:q0 + 128, :].rearrange("(g p) j -> p j g", p=16))
                    jw = sm.tile([128, 72], dt.uint16, tag="jw")
                    nc.vector.tensor_copy(jw[:], idxw[:])  # consume idxw
                    nc.vector.tensor_copy(ta[:], h16[:, q0:q0 + 128])
                else:
                    idxw16 = sm.tile([16, 72], dt.uint16, tag="iw16")
                    nc.sync.dma_start(
                        idxw16[:].rearrange("p (j g) -> p j g", j=K),
                        idx_d[q0:q0 + 128, :].rearrange("(g p) j -> p j g", p=16))
                    idxwf = sm.tile([16, 72], dt.float32, tag="iwf")
                    nc.scalar.activation(idxwf[:], idxw16[:], AF.Copy)
                    pr = psR.tile([128, 72], dt.float32, tag="pr")
                    nc.tensor.matmul(pr[:], repPs[:], idxwf[:],
                                     start=True, stop=True)
                    idxw = sm.tile([128, 72], dt.int16, tag="iw")
                    nc.scalar.activation(idxw[:], pr[:], AF.Copy)
                    gat = gth.tile([128, K * 128], dt.float32, tag="gat")
                    nc.gpsimd.ap_gather(
                        gat[:].rearrange("c (n d) -> c n d", d=1),
                        h32[:].rearrange("c (n d) -> c n d", d=1),
                        idxw[:], channels=128, num_elems=N, d=1,
                        num_idxs=K * 128)
                    # maxn over the 9 gathered slices (wide max tree, f32)
                    fa = mxp.tile([128, 512], dt.float32, tag="fa")
                    nc.vector.tensor_tensor(fa[:], gat[:, 0:512],
                                            gat[:, 512:1024], op=ALU.max)
                    fb = mxp.tile([128, 256], dt.float32, tag="fb")
                    nc.vector.tensor_tensor(fb[:], fa[:, 0:256], fa[:, 256:512],
                                            op=ALU.max)
                    fc = mxp.tile([128, 128], dt.float32, tag="fc")
                    nc.vector.tensor_tensor(fc[:], fb[:, 0:128], fb[:, 128:256],
                                            op=ALU.max)
                    nc.vector.tensor_tensor(ta[:], fc[:], gat[:, 1024:1152],
                                            op=ALU.max)
                # MaxRel conv: convpre = A @ h_q + W2 @ maxn  (A = W1 - W2)
                pc = psC.tile([128, 128], dt.float32, tag="pc")
                nc.tensor.matmul(pc[:], cwA[:], h16[:, q0:q0 + 128],
                                 start=True, stop=False)
                nc.tensor.matmul(pc[:], cw2[:], ta[:], start=False, stop=True)
                nc.scalar.activation(convpre[:, q0:q0 + 128], pc[:], AF.Identity,
                                     bias=convb[:], accum_out=cs1[:, i:i + 1])
                jc = sm.tile([128, 128], dt.float16, tag="jc")
                nc.scalar.activation(jc[:], convpre[:, q0:q0 + 128], AF.Square,
                                     accum_out=cs2[:, i:i + 1])

        # ---------- Phase D: BNc -> GELU -> fc2 -> BN2 -> +residual ----------
        totc = allreduce2(cs1, cs2)
        scc, bic = bnparams(totc[:, 0:1], totc[:, 1:2], bncg[:], bncb[:],
                            1.0 / (B * N))
        g16 = pers.tile([128, NQ], dt.float16)
        for cch in range(4):
            sl = slice(cch * 512, (cch + 1) * 512)
            nc.scalar.activation(g16[:, sl], convpre[:, sl], AF.Gelu,
                                 bias=bic, scale=scc)

        f2pre = pers.tile([128, NQ], dt.float32)
        fs1 = pers.tile([128, 4], dt.float32)
        fs2 = pers.tile([128, 4], dt.float32)
        with tc.tile_pool(name="fj", bufs=2) as fj, \
             tc.tile_pool(name="psF", bufs=2, space="PSUM") as psF:
            for cch in range(4):
                sl = slice(cch * 512, (cch + 1) * 512)
                pf = psF.tile([128, 512], dt.float32, tag="pf")
                nc.tensor.matmul(pf[:], fc2w[:], g16[:, sl], start=True, stop=True)
                nc.scalar.activation(f2pre[:, sl], pf[:], AF.Identity,
                                     bias=fc2b[:], accum_out=fs1[:, cch:cch + 1])
                jf = fj.tile([128, 512], dt.float16, tag="jf")
                nc.vector.scalar_tensor_tensor(jf[:], f2pre[:, sl], 1.0,
                                               f2pre[:, sl], op0=ALU.mult,
                                               op1=ALU.mult,
                                               accum_out=fs2[:, cch:cch + 1])

        totf = allreduce2(fs1, fs2)
        scf, bif = bnparams(totf[:, 0:1], totf[:, 1:2], bn2g[:], bn2bb[:],
                            1.0 / (B * N))
        outfm = pers.tile([128, NQ], dt.float32)
        for cch in range(4):
            sl = slice(cch * 512, (cch + 1) * 512)
            nc.scalar.activation(outfm[:, sl], f2pre[:, sl], AF.Identity,
                                 bias=bif, scale=scf)

        with tc.tile_pool(name="op", bufs=3) as op, \
             tc.tile_pool(name="psO", bufs=2, space="PSUM") as psO:
            for t in range(4):
                r0 = t * 512
                xr = op.tile([128, 512], dt.float32, tag="xr")
                nc.sync.dma_start(
                    xr[:].rearrange("p (j c) -> p j c", j=4),
                    x_res[r0:r0 + 512, :].rearrange("(j p) c -> p j c", p=128))
                po = psO.tile([128, 512], dt.float32, tag="po")
                for j in range(4):
                    q0 = r0 + j * 128
                    nc.tensor.transpose(po[:, j * 128:(j + 1) * 128],
                                        outfm[:, q0:q0 + 128], identF[:])
                ot = op.tile([128, 512], dt.float32, tag="ot")
                nc.vector.tensor_add(ot[:], po[:], xr[:])
                nc.sync.dma_start(
                    y[r0:r0 + 512, :].rearrange("(j p) c -> p j c", p=128),
                    ot[:].rearrange("p (j c) -> p j c", j=4))

        for p in (dram, pers, wpool):
            p.release()

    nc.compile()
    return nc


def _prep_weights(inputs):
    f32 = lambda a: np.ascontiguousarray(np.asarray(a), dtype=np.float32)
    f16 = lambda a: np.ascontiguousarray(np.asarray(a), dtype=np.float16)
    conv_w = np.asarray(inputs['conv_w'], np.float32)
    w1 = conv_w[:, :C]
    w2 = conv_w[:, C:]
    return {
        'fc1wT': f32(np.asarray(inputs['fc1_w'], np.float32).T),
        'fc1wN': f32(inputs['fc1_w']),
        'fc1_b': f32(inputs['fc1_b']),
        'bn1_g': f32(inputs['bn1_g']),
        'bn1_b': f32(inputs['bn1_b']),
        'cwAT16': f16((w1 - w2).T),
        'cw2T16': f16(w2.T),
        'conv_b': f32(inputs['conv_b']),
        'bnc_g': f32(inputs['bnc_g']),
        'bnc_b': f32(inputs['bnc_b']),
        'fc2wT16': f16(np.asarray(inputs['fc2_w'], np.float32).T),
        'fc2_b': f32(inputs['fc2_b']),
        'bn2_g': f32(inputs['bn2_g']),
        'bn2_b': f32(inputs['bn2_b']),
        'repP': f32(np.equal(np.arange(C)[None, :] % 16,
                             np.arange(16)[:, None]).astype(np.float32)),
    }


def _in_maps(inputs):
    x = np.ascontiguousarray(np.asarray(inputs['x']), dtype=np.float32)
    w = _prep_weights(inputs)
    maps = []
    for d in range(8):
        b, qoff = d // 4, (d % 4) * NQ
        m = dict(w)
        xr = np.roll(x[b], -qoff, axis=0)
        m['x_ownT'] = np.ascontiguousarray(xr.T)
        m['x_res'] = np.ascontiguousarray(x[b, qoff:qoff + NQ])
        m['x_oth'] = np.ascontiguousarray(x[1 - b])
        maps.append(m)
    return maps


def kernel(**inputs):
    from concourse import bass_utils

    if 'nc1' not in _CACHE:
        _CACHE['nc1'] = _build()
    nc1 = _CACHE['nc1']

    in_maps = _in_maps(inputs)
    r1 = bass_utils.run_bass_kernel_spmd(nc1, in_maps, core_ids=list(range(8)))
    _CACHE['last_res'] = r1

    out = np.empty((B, N, C), np.float32)
    for d in range(8):
        b, qoff = d // 4, (d % 4) * NQ
        out[b, qoff:qoff + NQ] = r1.results[d]['y']
    return out
